# revision 54
# baseline (speedup 1.0000x reference)
"""Trainium2 Bass kernel for nn_JpegCompression_patch (differentiable JPEG).

Algebraic reductions (verified vs reference in numpy):
 - The 3 RGB channels are identical, so Cb=Cr=128 exactly: chroma is a no-op
   and only the luma path matters (luma == input value).
 - pad(16x16, edge) + blockify + DCT + /(quant*factor) is one linear map
   W1 [196 -> 256] per image; dequant + IDCT + merge + crop is W2 [256 -> 196].
 - diff_round(t) = r + e^3 with r = rne(t), e = t - r, so the reconstructed
   (shifted-domain) pixel is y2 = W2 @ (r + e^3) and out = mn' + rng*(W2/255)@w2
   with mn' = mn + rng*128/255 (the /255 is folded into W2, so no separate s2
   scalar is ever computed).  The final clip to [mn, mn+rng] is dropped: only
   jpeg-overshoot pixels differ (measured rel err 5.4e-4 vs 3.7e-5 clipped,
   tolerance 2e-2; measured on-device 1.37e-3 end to end).

Per pair of 128-image groups ("flipped" stage-1 orientation):
  GPSIMD normalize u=(x-mn')/rng (tensor_scalar is the only elementwise op the
  TRN2 Pool engine accepts) -> PE transpose u (f32r, two 128-wide px windows
  0..127 / 68..195 so every PSUM row is written; W1's second k-chunk is
  zero-padded to kill the 60-row overlap) -> one ACT copy PSUM->SBUF per pair
  -> PE matmul, stationary W1 chunk, moving XT (f32r, free 256 = 1 cyc/row) ->
  QT[c, img] in PSUM -> DVE custom op JPEG_QERR2 (w2 = rne(t) + e^3 by
  magic-number round, one pass per 2 pairs) -> PE matmul, stationary w2[c,img]
  f32r, moving W2 f32r zero-padded to 256 cols (16/32-bit operand mixing is
  rejected by the compiler; f32r moving needs free >= 256 for full rate) ->
  CORR[img, px] -> ACT Identity activation out = rng*CORR + mn' (per-partition
  scale/bias APs) -> half-supertile output DMAs.
Min/max run on DVE TensorReduce in half-supertile batches: TRN2's Pool engine
cannot reduce the free axis and tensor_tensor_reduce crashes the exec unit.
Inputs stream as 8 half-supertile DMAs issued before the constants; outputs
issue from SP after all inputs.  Engine busy per core: DVE ~25us (reduces 14 +
QERR 9.5), ACT ~22us (copies + final affines), PE ~20us, DMA ~19.3us (the
6.4MB in+out roofline), Pool ~12us (normalize).

Sharding: pure data parallel over the fused 32*1024 image axis, 4096 imgs/core.
TimelineSim: 42374 ns/core (baseline 55431).
"""

import os

import numpy as np
import ml_dtypes

import concourse.bass as bass
import concourse.mybir as mybir
from concourse.bacc import Bacc
from concourse.tile import TileContext
from concourse import bass_utils
from concourse.dve_ops import (
    OPS,
    DveOp,
    _SUB_OPCODE_FOR_NAME,
    _CUSTOM_DVE_ROW_BASE,
    CUSTOM_DVE_SPECS,
)
from concourse.dve_spec import Spec, lower, Src0, C2, sq, _has_src1
from concourse.dve_uop import DveOpSpec
from concourse.dve_table_gen import dve_ver_for

N_CORES = 8
TOT_IMGS = 32 * 1024
IMGS_PER_CORE = TOT_IMGS // N_CORES  # 4096
PX = 196
GS = 8  # image-groups (of 128) per supertile
NSUP = IMGS_PER_CORE // (128 * GS)  # 4 supertiles
MAGIC = 12582912.0  # 1.5 * 2**23: (x + M) - M == rne(x) for |x| < 2**22
FMAX = 3.4e38

N_RED_D = 2   # groups per supertile whose min/max run on DVE (rest: Pool scan)
N_FIN_D = 0   # groups per supertile whose final affine runs on DVE (rest: ACT)

F32 = mybir.dt.float32
F32R = mybir.dt.float32r
BF16 = mybir.dt.bfloat16


# ---------------- custom DVE op ----------------
def _register(name: str, spec: Spec) -> DveOp:
    if name in _SUB_OPCODE_FOR_NAME:
        for op in OPS:
            if op.name == name:
                return op
    row = _CUSTOM_DVE_ROW_BASE + len(OPS)
    assert row < 0x20, "custom DVE opcode rows exhausted"
    ver = dve_ver_for("TRN2")
    uops = lower(spec, ver=ver)
    sha = DveOpSpec(name=name, opcode=row, uops=uops, rd1_en=_has_src1(spec)).sha(ver)
    _SUB_OPCODE_FOR_NAME[name] = row
    op = DveOp(name, spec, subdim=False, uops_sha={ver: sha})
    OPS.append(op)
    CUSTOM_DVE_SPECS[name] = spec
    return op


def _qerr2_ref(in0, in1, s0, s1, imm2):
    x = in0.astype(np.float32)
    m = np.float32(imm2)
    r = (x + m) - m
    e = x - r
    return r + (e * e) * e


# w2 = rne(t) + e^3, e = t - rne(t); in0 = t, imm2 = MAGIC
_t = Src0 + C2
_r = _t - C2
_e = Src0 - _r
QERR2_OP = _register("JPEG_QERR2", Spec(body=_r + sq(_e) * _e, reference=_qerr2_ref))


# ---------------- constant matrices ----------------
def _build_mats():
    i = np.arange(8, dtype=np.float64)
    T = (
        np.cos((2 * i[:, None, None, None] + 1) * i[None, None, :, None] * np.pi / 16)
        * np.cos((2 * i[None, :, None, None] + 1) * i[None, None, None, :] * np.pi / 16)
    )
    alpha = np.ones(8)
    alpha[0] = 1.0 / np.sqrt(2.0)
    dct_scale = np.outer(alpha, alpha) * 0.25
    idct_alpha = np.outer(alpha, alpha)
    ytab = np.array(
        [
            [16, 11, 10, 16, 24, 40, 51, 61],
            [12, 12, 14, 19, 26, 58, 60, 55],
            [14, 13, 16, 24, 40, 57, 69, 56],
            [14, 17, 22, 29, 51, 87, 80, 62],
            [18, 22, 37, 56, 68, 109, 103, 77],
            [24, 35, 55, 64, 81, 104, 113, 92],
            [49, 64, 78, 87, 103, 121, 120, 101],
            [72, 92, 95, 98, 112, 100, 103, 99],
        ],
        dtype=np.float64,
    )
    factor = (200.0 - 2.0 * 99.0) / 100.0  # quality=99 -> 0.02
    d = ytab * factor

    pmap = np.clip(np.arange(16) - 1, 0, 13)  # padded idx -> orig idx (edge pad)

    # W1[pixel, (br,bc,u,v)]: u-domain (-0.5..0.5) pixel -> quantized DCT
    W1 = np.zeros((14, 14, 2, 2, 8, 8))
    for br in range(2):
        for bc in range(2):
            for x in range(8):
                for y in range(8):
                    W1[pmap[8 * br + x], pmap[8 * bc + y], br, bc, :, :] += (
                        dct_scale * T[x, y, :, :]
                    )
    W1 = (255.0 * W1 / d[None, None, None, None, :, :]).reshape(PX, 256)

    # W2[(br,bc,u,v), pixel]: (r + e^3) -> reconstructed shifted pixel
    W2 = np.zeros((2, 2, 8, 8, 14, 14))
    for r in range(14):
        for c in range(14):
            br, x = divmod(r + 1, 8)
            bc, y = divmod(c + 1, 8)
            W2[br, bc, :, :, r, c] = 0.25 * idct_alpha * T[x, y, :, :] * d
    W2 = W2.reshape(256, PX)

    w1a = W1[0:128, :].astype(np.float32)  # px 0..127
    # px window 68..195 (rows 0..59 are px 68..127, already in w1a -> zero)
    w1b = np.zeros((128, 256), np.float32)
    w1b[60:128, :] = W1[128:196, :]
    # f32r, zero-padded to 256 cols: the f32r moving operand needs out free
    # size >= 256 for the 1 cycle/row PE path (and bf16 can't mix with f32r)
    w2a = np.zeros((128, 256), np.float32)
    w2a[:, 0:PX] = W2[0:128, :] / 255.0
    w2b = np.zeros((128, 256), np.float32)
    w2b[:, 0:PX] = W2[128:256, :] / 255.0
    return w1a, w1b, w2a, w2b


# ---------------- bass program ----------------
def build_nc():
    nc = Bacc("TRN2", target_bir_lowering=False, debug=False)
    x_d = nc.dram_tensor("x", [IMGS_PER_CORE, PX], F32, kind="ExternalInput")
    w1a_d = nc.dram_tensor("w1a", [128, 256], F32R, kind="ExternalInput")
    w1b_d = nc.dram_tensor("w1b", [128, 256], F32R, kind="ExternalInput")
    w2a_d = nc.dram_tensor("w2a", [128, 256], F32R, kind="ExternalInput")
    w2b_d = nc.dram_tensor("w2b", [128, 256], F32R, kind="ExternalInput")
    idf_d = nc.dram_tensor("idf", [128, 128], F32R, kind="ExternalInput")
    y_d = nc.dram_tensor("y", [IMGS_PER_CORE, PX], F32, kind="ExternalOutput")

    AL = mybir.AluOpType
    AX = mybir.AxisListType
    ACTF = mybir.ActivationFunctionType
    NSC = GS - N_RED_D  # pool-scanned groups per supertile
    R = 128 * GS

    with TileContext(nc) as tc:
        with (
            tc.tile_pool(name="const", bufs=1) as cpool,
            tc.tile_pool(name="xp", bufs=NSUP) as xpool,
            tc.tile_pool(name="vp", bufs=4) as vpool,
            tc.tile_pool(name="xtp", bufs=4) as xtpool,
            tc.tile_pool(name="wp", bufs=4) as wpool,
            tc.tile_pool(name="yp", bufs=3) as ypool,
            tc.tile_pool(name="sm", bufs=3) as smpool,
            tc.tile_pool(name="scp", bufs=4) as scpool,
            tc.tile_pool(name="pt_ps", bufs=2, space="PSUM") as ptpool,
            tc.tile_pool(name="q_ps", bufs=2, space="PSUM") as qpool,
            tc.tile_pool(name="c_ps", bufs=2, space="PSUM") as cpspool,
        ):
            # first input supertile DMA before anything else, consts next,
            # then the remaining inputs (SP queue is in-order; outs last)
            X4s = []
            H = GS // 2
            RH = 128 * H

            def dma_in(T):
                X4 = xpool.tile([128, GS, PX], F32, tag="x")
                for h in range(2):
                    r0 = R * T + RH * h
                    nc.sync.dma_start(
                        X4[:, h * H : (h + 1) * H, :],
                        x_d[r0 : r0 + RH, :].rearrange("(g p) c -> p g c", g=H),
                    )
                X4s.append(X4)

            dma_in(0)
            w1a = cpool.tile([128, 256], F32R, tag="w1a")
            nc.sync.dma_start(w1a, w1a_d[:, :])
            w1b = cpool.tile([128, 256], F32R, tag="w1b")
            nc.sync.dma_start(w1b, w1b_d[:, :])
            w2a = cpool.tile([128, 256], F32R, tag="w2a")
            nc.sync.dma_start(w2a, w2a_d[:, :])
            w2b = cpool.tile([128, 256], F32R, tag="w2b")
            nc.sync.dma_start(w2b, w2b_d[:, :])
            idf = cpool.tile([128, 128], F32R, tag="idf")
            nc.sync.dma_start(idf, idf_d[:, :])
            for T in range(1, NSUP):
                dma_in(T)

            stats = {}

            def emit_smalls(T, si):
                """derived scalars: si 0 -> g0..1, 1 -> g2..7.  s2 is folded
                into W2 (/255): finals scale by rng; mn' = mn + rng*128/255."""
                mn4, mx4, rng4, rcp4, mnp4 = stats[T]
                sl = slice(0, 4) if si == 0 else slice(4, GS)
                mns, mxs = mn4[:, sl], mx4[:, sl]
                nc.vector.scalar_tensor_tensor(
                    rng4[:, sl], mxs, 1e-5, mns, AL.add, AL.subtract
                )
                nc.vector.reciprocal(rcp4[:, sl], rng4[:, sl])
                nc.vector.scalar_tensor_tensor(
                    mnp4[:, sl], rng4[:, sl], 128.0 / 255.0, mns, AL.mult, AL.add
                )

            def emit_stats(T):
                """per-image min/max: DVE TensorReduce in two half-supertile
                batches (GPSIMD cannot reduce the free axis on TRN2, and
                tensor_tensor_reduce crashes the exec unit)."""
                X4 = X4s[T]
                mn4 = smpool.tile([128, GS], F32, tag="mn")
                mx4 = smpool.tile([128, GS], F32, tag="mx")
                rng4 = smpool.tile([128, GS], F32, tag="rng")
                rcp4 = smpool.tile([128, GS], F32, tag="rcp")
                mnp4 = smpool.tile([128, GS], F32, tag="mnp")
                Hh = GS // 2
                for h in range(2):
                    sl = slice(h * Hh, (h + 1) * Hh)
                    nc.vector.tensor_reduce(
                        mn4[:, sl], X4[:, sl, :], axis=AX.X, op=AL.min
                    )
                    nc.vector.tensor_reduce(
                        mx4[:, sl], X4[:, sl, :], axis=AX.X, op=AL.max
                    )
                stats[T] = (mn4, mx4, rng4, rcp4, mnp4)

            emit_stats(0)

            for T in range(NSUP):
                X4 = X4s[T]
                _, _, rng4, rcp4, mnp4 = stats[T]
                Y4 = ypool.tile([128, GS, PX], F32, tag="y")
                QTs, Ws = [None, None], [None, None]

                def fwd(p):
                    ga, gb = 2 * p, 2 * p + 1
                    V = vpool.tile([128, 2, PX], F32R, tag="v")
                    for j, g in ((0, ga), (1, gb)):
                        nc.gpsimd.tensor_scalar(
                            V[:, j, :], X4[:, g, :],
                            mnp4[:, g : g + 1], rcp4[:, g : g + 1],
                            AL.subtract, AL.mult,
                        )
                    PT = ptpool.tile([128, 512], F32R, tag="pt")
                    nc.tensor.transpose(PT[:, 0:128], V[:, 0, 0:128], idf)
                    nc.tensor.transpose(PT[:, 128:256], V[:, 1, 0:128], idf)
                    nc.tensor.transpose(PT[:, 256:384], V[:, 0, 68:196], idf)
                    nc.tensor.transpose(PT[:, 384:512], V[:, 1, 68:196], idf)
                    XT = xtpool.tile([128, 512], F32R, tag="xt")
                    nc.scalar.copy(XT, PT)

                    pp, ph = divmod(p, 2)
                    if ph == 0:
                        QTn = qpool.tile([128, 1024], F32, tag="q")
                        QTs[pp] = QTn
                    QT = QTs[pp]
                    q0 = 512 * ph
                    nc.tensor.matmul(QT[:, q0 : q0 + 256], w1a[:, 0:128],
                                     XT[:, 0:256], start=True, stop=False)
                    nc.tensor.matmul(QT[:, q0 : q0 + 256], w1b[:, 0:128],
                                     XT[:, 256:512], start=False, stop=True)
                    nc.tensor.matmul(QT[:, q0 + 256 : q0 + 512], w1a[:, 128:256],
                                     XT[:, 0:256], start=True, stop=False)
                    nc.tensor.matmul(QT[:, q0 + 256 : q0 + 512], w1b[:, 128:256],
                                     XT[:, 256:512], start=False, stop=True)

                def qerr(pp, split=False):
                    Wn = wpool.tile([128, 1024], F32R, tag="w")
                    if split:
                        # drain phase: per-pair halves so bwd can start earlier
                        nc.vector._custom_dve(
                            QERR2_OP, out=Wn[:, 0:512], in0=QTs[pp][:, 0:512],
                            imm2=MAGIC,
                        )
                        nc.vector._custom_dve(
                            QERR2_OP, out=Wn[:, 512:1024],
                            in0=QTs[pp][:, 512:1024], imm2=MAGIC,
                        )
                    else:
                        nc.vector._custom_dve(QERR2_OP, out=Wn, in0=QTs[pp], imm2=MAGIC)
                    Ws[pp] = Wn

                def bwd(p):
                    pp, ph = divmod(p, 2)
                    W = Ws[pp]
                    ga, gb = 2 * p, 2 * p + 1
                    CORR = cpspool.tile([128, 2, 256], F32, tag="corr")
                    q0 = 512 * ph
                    for j, g in ((0, ga), (1, gb)):
                        nc.tensor.matmul(
                            CORR[:, j, :],
                            W[:, q0 + 128 * j : q0 + 128 * j + 128],
                            w2a, start=True, stop=False,
                        )
                        nc.tensor.matmul(
                            CORR[:, j, :],
                            W[:, q0 + 256 + 128 * j : q0 + 256 + 128 * j + 128],
                            w2b, start=False, stop=True,
                        )
                    for j, g in ((0, ga), (1, gb)):
                        # in the drain phase (last supertile's second half) DVE
                        # is idle: run half the finals there to cut the tail
                        if T == NSUP - 1 and p >= 2 and j == 0:
                            nc.vector.tensor_scalar(
                                Y4[:, g, :], CORR[:, j, 0:PX],
                                rng4[:, g : g + 1], mnp4[:, g : g + 1],
                                AL.mult, AL.add,
                            )
                        else:
                            nc.scalar.activation(
                                Y4[:, g, :], CORR[:, j, 0:PX], ACTF.Identity,
                                bias=mnp4[:, g : g + 1], scale=rng4[:, g : g + 1],
                            )

                emit_smalls(T, 0)
                fwd(0)
                fwd(1)
                emit_smalls(T, 1)
                qerr(0, split=True)
                fwd(2)
                bwd(0)
                fwd(3)
                bwd(1)
                if T + 1 < NSUP:
                    emit_stats(T + 1)
                qerr(1, split=(T == NSUP - 1))
                r0 = R * T
                yo0 = y_d[r0 : r0 + RH, :].rearrange("(g p) c -> p g c", g=H)
                nc.sync.dma_start(yo0, Y4[:, 0:H, :])
                bwd(2)
                if T == NSUP - 1:
                    # drain: quarter the last output so g4..5 ship before bwd(3)
                    RQ = 128 * 2
                    yq2 = y_d[r0 + RH : r0 + RH + RQ, :].rearrange(
                        "(g p) c -> p g c", g=2
                    )
                    nc.sync.dma_start(yq2, Y4[:, 4:6, :])
                    bwd(3)
                    yq3 = y_d[r0 + RH + RQ : r0 + R, :].rearrange(
                        "(g p) c -> p g c", g=2
                    )
                    nc.sync.dma_start(yq3, Y4[:, 6:8, :])
                else:
                    bwd(3)
                    yo1 = y_d[r0 + RH : r0 + R, :].rearrange("(g p) c -> p g c", g=H)
                    nc.sync.dma_start(yo1, Y4[:, H:GS, :])
    nc.finalize()
    return nc


_CACHE: dict = {}


def kernel(x):
    x = np.ascontiguousarray(np.asarray(x, dtype=np.float32))
    B, C, H, Wd = x.shape
    shards = x.reshape(N_CORES, IMGS_PER_CORE, PX)

    if "nc" not in _CACHE:
        _CACHE["nc"] = build_nc()
        w1a, w1b, w2a, w2b = _build_mats()
        _CACHE["consts"] = (w1a, w1b, w2a, w2b, np.eye(128, dtype=np.float32))
    nc = _CACHE["nc"]
    w1a, w1b, w2a, w2b, idf = _CACHE["consts"]
    in_maps = [
        {
            "x": np.ascontiguousarray(shards[i]),
            "w1a": w1a,
            "w1b": w1b,
            "w2a": w2a,
            "w2b": w2b,
            "idf": idf,
        }
        for i in range(N_CORES)
    ]
    res = bass_utils.run_bass_kernel_spmd(
        nc,
        in_maps,
        core_ids=list(range(N_CORES)),
        trace=bool(os.environ.get("KTRACE")),
    )
    if res.exec_time_ns is not None:
        print(f"[kernel] HW exec time: {res.exec_time_ns} ns")
        if res.instructions_and_trace is not None:
            print(f"[kernel] trace: {res.instructions_and_trace[1]}")
    out = np.stack([r["y"] for r in res.results], 0).reshape(B, C, H, Wd)
    return out
